# revision 22
# baseline (speedup 1.0000x reference)
"""Trainium2 Bass kernel for nn_KernelShiftedPrediction (v8: fp16 key scheme,
hoisted prefetch + pipelined decode with PE reconstruct).

Per pixel, over 9 shifts, pick the shifted `predicted` value minimizing
|target - candidate|. Track two fp16 "keys" = fp16(d), d = T - C:

 - kP = unsigned-int16 min over fp16 bits of d (best d>=0 candidate)
 - kN = signed-int16 min over the same bits (best d<0 candidate)

Both are plain 16-bit tensor_tensor mins (2x DVE mode). Decode per
image: compare magnitudes, select winner dw (fp16), reconstruct
C = I@T - I16@dw on the PE, ACT-copy out of PSUM, DMA out.

Software pipeline per iteration s:
  - emit DMA loads for image s+1 first (sync dispatches them before
    any instruction gated on slow producers),
  - 9 shifts of image s: PE d=I@T-I@C (fp32r), ACT fp16 convert
    (PSUM->SBUF), DVE kP/kN min updates,
  - decode of image s-1 interleaved at shift 4 (DVE ops) and shift 6
    (PE reconstruct + ACT out-copy + DMA store).

Accuracy: comparisons are fp16 (and fp32r ~11-bit operand rounding);
near-tie mispicks give rel_err ~1.24e-2 on the contest data (gate
2e-2); off-tie value error ~|d|*2^-11.
"""
import sys

sys.path.insert(0, "/opt/trn_rl_repo")

import numpy as np

S, B, H, W = 10, 8, 512, 512
CH = 128          # chunk rows (partitions)
NCH = H // CH     # 4 segments per image, side by side in the free dim
SEG = W + 2       # per-segment width in padded view tiles
FREE_T = NCH * W      # 2048
FREE_P = NCH * SEG    # 2056
PADVAL = 1.0e30

SHIFTS = [(0, 0), (-1, -1), (-1, 0), (-1, 1), (0, -1), (0, 1), (1, -1),
          (1, 0), (1, 1)]

_CACHE = {}


def _build_nc():
    import concourse.bacc as bacc
    import concourse.mybir as mybir
    from concourse.tile import TileContext

    F32 = mybir.dt.float32
    F32R = mybir.dt.float32r
    F16 = mybir.dt.float16
    U16 = mybir.dt.uint16
    I16 = mybir.dt.int16
    OP = mybir.AluOpType

    nc = bacc.Bacc("TRN2", target_bir_lowering=False, debug=False, num_devices=B)
    pred = nc.declare_dram_parameter("pred", [S, H, W], F32R, isOutput=False)
    targ = nc.declare_dram_parameter("targ", [S, H, W], F32R, isOutput=False)
    eye2 = nc.declare_dram_parameter("eye2", [128, 768], F32R, isOutput=False)
    neye16 = nc.declare_dram_parameter("neye16", [128, 128], U16, isOutput=False)
    out = nc.declare_dram_parameter("out", [S, H, W], F32, isOutput=True)

    with TileContext(nc) as tc:
        with (
            tc.tile_pool(name="cst", bufs=1) as cst,
            tc.tile_pool(name="io", bufs=3) as io,
            tc.tile_pool(name="hp", bufs=3) as hp,
            tc.tile_pool(name="kp", bufs=2) as kp,
            tc.tile_pool(name="dc", bufs=2) as dc,
            tc.tile_pool(name="ps", bufs=2, space="PSUM") as psp,
        ):
            eye = cst.tile([128, 768], F32R)
            nc.sync.dma_start(out=eye[:, :], in_=eye2[:, :])
            W_I = eye[:, 0:128]       # identity
            W_N = eye[:, 128:256]     # -identity
            W_NU = eye[:, 256:384]    # -shifted identity: out[m] = -C[m-1]
            W_ND = eye[:, 384:512]    # -shifted identity: out[m] = -C[m+1]
            W_EU = eye[:, 512:640]    # out[0]   = -rhs[127]
            W_ED = eye[:, 640:768]    # out[127] = -rhs[0]
            ney = cst.tile([128, 128], U16)
            nc.sync.dma_start(out=ney[:, :], in_=neye16[:, :])
            W_N16 = ney[:, :].bitcast(F16)   # -identity fp16
            PADT = cst.tile([CH, W], F32R)
            nc.vector.memset(PADT[:, :].bitcast(F32), PADVAL)

            def emit_loads(s):
                T = io.tile([CH, FREE_T], F32R, tag="T")
                PC = io.tile([CH, FREE_P], F32R, tag="PC")
                ap = PC[:, :].bitcast(F32).rearrange("p (g e) -> p g e", g=NCH)
                nc.gpsimd.memset(ap[:, :, 0:SEG:SEG - 1], PADVAL)
                for g in range(NCH):
                    r0 = g * CH
                    cs = g * SEG + 1
                    nc.sync.dma_start(
                        out=T[:, g * W : (g + 1) * W], in_=targ[s, r0 : r0 + CH, :]
                    )
                    nc.sync.dma_start(
                        out=PC[:, cs : cs + W], in_=pred[s, r0 : r0 + CH, :]
                    )
                return (T, PC)

            def emit_decode_a(kP, kN):
                # dw = magnitude-smaller of kP / kN  (all u16 DVE ops)
                aP = dc.tile([CH, FREE_T], U16, tag="aP")
                aN = dc.tile([CH, FREE_T], U16, tag="aN")
                m = dc.tile([CH, FREE_T], U16, tag="m")
                dw = dc.tile([CH, FREE_T], U16, tag="dw")
                nc.vector.tensor_scalar(
                    out=aP[:, :], in0=kP[:, :], scalar1=0x7FFF, scalar2=None,
                    op0=OP.bitwise_and,
                )
                nc.vector.tensor_scalar(
                    out=aN[:, :], in0=kN[:, :].bitcast(U16), scalar1=0x7FFF,
                    scalar2=None, op0=OP.bitwise_and,
                )
                nc.vector.tensor_tensor(m[:, :], aP[:, :], aN[:, :], OP.is_lt)
                nc.vector.tensor_copy(dw[:, :], kN[:, :].bitcast(U16))
                nc.vector.copy_predicated(dw[:, :], m[:, :], kP[:, :])
                return dw

            def emit_decode_b(T, dw, s):
                # reconstruct C = I@T - I16@dw on PE; ACT copy out; DMA store
                bv = dc.tile([CH, FREE_T], F32, tag="bv")
                ps2 = psp.tile([CH, FREE_T], F32, tag="ps")
                for g in range(NCH):
                    sl = slice(g * W, (g + 1) * W)
                    nc.tensor.matmul(
                        ps2[:, sl], W_I, T[:, sl], start=True, stop=False
                    )
                    nc.tensor.matmul(
                        ps2[:, sl], W_N16, dw[:, sl].bitcast(F16),
                        start=False, stop=True,
                    )
                nc.scalar.copy(bv[:, :], ps2[:, :])
                for g in range(NCH):
                    nc.sync.dma_start(
                        out=out[s, g * CH : (g + 1) * CH, :],
                        in_=bv[:, g * W : (g + 1) * W],
                    )

            tiles = {0: emit_loads(0)}
            pending = None
            for s in range(S):
                if s + 1 < S:
                    tiles[s + 1] = emit_loads(s + 1)
                T, PC = tiles.pop(s)

                def pc_seg(y, g):
                    return PC[:, g * SEG + 1 + y : g * SEG + 1 + y + W]

                kP = kp.tile([CH, FREE_T], U16, tag="kP")
                kN = kp.tile([CH, FREE_T], I16, tag="kN")

                for si, (x, y) in enumerate(SHIFTS):
                    if si == 4 and pending is not None:
                        T0, kP0, kN0, s0 = pending
                        dw0 = emit_decode_a(kP0, kN0)
                    if si == 6 and pending is not None:
                        emit_decode_b(T0, dw0, s0)
                        pending = None
                    h = hp.tile([CH, FREE_T], U16, tag="h")
                    ps = psp.tile([CH, FREE_T], F32, tag="ps")
                    for g in range(NCH):
                        sl = slice(g * W, (g + 1) * W)
                        nc.tensor.matmul(
                            ps[:, sl], W_I, T[:, sl], start=True, stop=False
                        )
                        if x == 0:
                            nc.tensor.matmul(
                                ps[:, sl], W_N, pc_seg(y, g),
                                start=False, stop=True,
                            )
                        elif x == -1:
                            nc.tensor.matmul(
                                ps[:, sl], W_NU, pc_seg(y, g),
                                start=False, stop=False,
                            )
                            nc.tensor.matmul(
                                ps[:, sl], W_EU,
                                pc_seg(y, g - 1) if g > 0 else PADT[:, :],
                                start=False, stop=True,
                            )
                        else:
                            nc.tensor.matmul(
                                ps[:, sl], W_ND, pc_seg(y, g),
                                start=False, stop=False,
                            )
                            nc.tensor.matmul(
                                ps[:, sl], W_ED,
                                pc_seg(y, g + 1) if g < NCH - 1 else PADT[:, :],
                                start=False, stop=True,
                            )
                    nc.scalar.copy(h[:, :].bitcast(F16), ps[:, :])
                    if si == 0:
                        nc.vector.tensor_copy(kP[:, :], h[:, :])
                        nc.vector.tensor_copy(kN[:, :].bitcast(U16), h[:, :])
                    else:
                        nc.vector.tensor_tensor(
                            kP[:, :], h[:, :], kP[:, :], OP.min
                        )
                        nc.vector.tensor_tensor(
                            kN[:, :], h[:, :].bitcast(I16), kN[:, :], OP.min
                        )

                pending = (T, kP, kN, s)

            dw_l = emit_decode_a(pending[1], pending[2])
            emit_decode_b(pending[0], dw_l, pending[3])

    nc.finalize()
    return nc


def _get_nc():
    if "nc" not in _CACHE:
        _CACHE["nc"] = _build_nc()
    return _CACHE["nc"]


def kernel(predicted, target, mask=None, _want_results_obj=False, _trace=False):
    """predicted [S,B,H,W], target [B,S,H,W] -> [S,B,H,W] (mask unused)."""
    from concourse.bass_utils import run_bass_kernel_spmd

    nc = _get_nc()
    eye = np.eye(128, dtype=np.float32)
    e_up = np.eye(128, k=1, dtype=np.float32)     # [k, m]=1 at k=m-1
    e_dn = np.eye(128, k=-1, dtype=np.float32)    # [k, m]=1 at k=m+1
    e_bu = np.zeros((128, 128), dtype=np.float32)
    e_bu[127, 0] = 1.0                             # out[0] = rhs[127]
    e_bd = np.zeros((128, 128), dtype=np.float32)
    e_bd[0, 127] = 1.0                             # out[127] = rhs[0]
    eye2 = np.concatenate([eye, -eye, -e_up, -e_dn, -e_bu, -e_bd], axis=1)
    neye16 = np.where(np.eye(128, dtype=bool), 0xBC00, 0).astype(np.uint16)
    in_maps = []
    for b in range(B):
        in_maps.append(
            {
                "pred": np.ascontiguousarray(predicted[:, b]),
                "targ": np.ascontiguousarray(target[b]),
                "eye2": eye2,
                "neye16": neye16,
            }
        )
    res = run_bass_kernel_spmd(nc, in_maps, list(range(B)), trace=_trace)
    outp = np.stack([res.results[b]["out"] for b in range(B)], axis=1)
    if _want_results_obj:
        return outp, res
    return outp


# revision 23
# speedup vs baseline: 1.0784x; 1.0784x over previous
"""Trainium2 Bass kernel for nn_KernelShiftedPrediction (v8: fp16 key scheme,
hoisted prefetch + pipelined decode with PE reconstruct).

Per pixel, over 9 shifts, pick the shifted `predicted` value minimizing
|target - candidate|. Track two fp16 "keys" = fp16(d), d = T - C:

 - kP = unsigned-int16 min over fp16 bits of d (best d>=0 candidate)
 - kN = signed-int16 min over the same bits (best d<0 candidate)

Both are plain 16-bit tensor_tensor mins (2x DVE mode). Decode per
image: compare magnitudes, select winner dw (fp16), reconstruct
C = I@T - I16@dw on the PE, ACT-copy out of PSUM, DMA out.

Software pipeline per iteration s:
  - emit DMA loads for image s+1 first (sync dispatches them before
    any instruction gated on slow producers),
  - 9 shifts of image s: PE d=I@T-I@C (fp32r), ACT fp16 convert
    (PSUM->SBUF), DVE kP/kN min updates,
  - decode of image s-1 interleaved at shift 4 (DVE ops) and shift 6
    (PE reconstruct + ACT out-copy + DMA store).

Accuracy: comparisons are fp16 (and fp32r ~11-bit operand rounding);
near-tie mispicks give rel_err ~1.24e-2 on the contest data (gate
2e-2); off-tie value error ~|d|*2^-11.
"""
import sys

sys.path.insert(0, "/opt/trn_rl_repo")

import numpy as np

S, B, H, W = 10, 8, 512, 512
CH = 128          # chunk rows (partitions)
NCH = H // CH     # 4 segments per image, side by side in the free dim
SEG = W + 2       # per-segment width in padded view tiles
FREE_T = NCH * W      # 2048
FREE_P = NCH * SEG    # 2056
PADVAL = 1.0e30

SHIFTS = [(0, 0), (-1, -1), (-1, 0), (-1, 1), (0, -1), (0, 1), (1, -1),
          (1, 0), (1, 1)]

_CACHE = {}


def _build_nc():
    import concourse.bacc as bacc
    import concourse.mybir as mybir
    from concourse.tile import TileContext

    F32 = mybir.dt.float32
    F32R = mybir.dt.float32r
    F16 = mybir.dt.float16
    U16 = mybir.dt.uint16
    I16 = mybir.dt.int16
    OP = mybir.AluOpType

    nc = bacc.Bacc("TRN2", target_bir_lowering=False, debug=False, num_devices=B)
    pred = nc.declare_dram_parameter("pred", [S, H, W], F32R, isOutput=False)
    targ = nc.declare_dram_parameter("targ", [S, H, W], F32R, isOutput=False)
    eye2 = nc.declare_dram_parameter("eye2", [128, 768], F32R, isOutput=False)
    neye16 = nc.declare_dram_parameter("neye16", [128, 128], U16, isOutput=False)
    out = nc.declare_dram_parameter("out", [S, H, W], F32, isOutput=True)

    with TileContext(nc) as tc:
        with (
            tc.tile_pool(name="cst", bufs=1) as cst,
            tc.tile_pool(name="io", bufs=6) as io,
            tc.tile_pool(name="hp", bufs=4) as hp,
            tc.tile_pool(name="kp", bufs=4) as kp,
            tc.tile_pool(name="dc", bufs=2) as dc,
            tc.tile_pool(name="ps", bufs=2, space="PSUM") as psp,
        ):
            eye = cst.tile([128, 768], F32R)
            nc.sync.dma_start(out=eye[:, :], in_=eye2[:, :])
            W_I = eye[:, 0:128]       # identity
            W_N = eye[:, 128:256]     # -identity
            W_NU = eye[:, 256:384]    # -shifted identity: out[m] = -C[m-1]
            W_ND = eye[:, 384:512]    # -shifted identity: out[m] = -C[m+1]
            W_EU = eye[:, 512:640]    # out[0]   = -rhs[127]
            W_ED = eye[:, 640:768]    # out[127] = -rhs[0]
            ney = cst.tile([128, 128], U16)
            nc.sync.dma_start(out=ney[:, :], in_=neye16[:, :])
            W_N16 = ney[:, :].bitcast(F16)   # -identity fp16
            PADT = cst.tile([CH, W], F32R)
            nc.vector.memset(PADT[:, :].bitcast(F32), PADVAL)

            def emit_loads(s):
                T = io.tile([CH, FREE_T], F32R, tag="T")
                PC = io.tile([CH, FREE_P], F32R, tag="PC")
                ap = PC[:, :].bitcast(F32).rearrange("p (g e) -> p g e", g=NCH)
                nc.gpsimd.memset(ap[:, :, 0:SEG:SEG - 1], PADVAL)
                for g in range(NCH):
                    r0 = g * CH
                    cs = g * SEG + 1
                    nc.sync.dma_start(
                        out=T[:, g * W : (g + 1) * W], in_=targ[s, r0 : r0 + CH, :]
                    )
                    nc.sync.dma_start(
                        out=PC[:, cs : cs + W], in_=pred[s, r0 : r0 + CH, :]
                    )
                return (T, PC)

            def emit_decode_a(kP, kN):
                # dw = magnitude-smaller of kP / kN  (all u16 DVE ops)
                aP = dc.tile([CH, FREE_T], U16, tag="aP")
                aN = dc.tile([CH, FREE_T], U16, tag="aN")
                m = dc.tile([CH, FREE_T], U16, tag="m")
                dw = dc.tile([CH, FREE_T], U16, tag="dw")
                nc.vector.tensor_scalar(
                    out=aP[:, :], in0=kP[:, :], scalar1=0x7FFF, scalar2=None,
                    op0=OP.bitwise_and,
                )
                nc.vector.tensor_scalar(
                    out=aN[:, :], in0=kN[:, :].bitcast(U16), scalar1=0x7FFF,
                    scalar2=None, op0=OP.bitwise_and,
                )
                nc.vector.tensor_tensor(m[:, :], aP[:, :], aN[:, :], OP.is_lt)
                nc.vector.tensor_copy(dw[:, :], kN[:, :].bitcast(U16))
                nc.vector.copy_predicated(dw[:, :], m[:, :], kP[:, :])
                return dw

            def emit_decode_b(T, dw, s):
                # reconstruct C = I@T - I16@dw on PE; ACT copy out; DMA store
                bv = dc.tile([CH, FREE_T], F32, tag="bv")
                ps2 = psp.tile([CH, FREE_T], F32, tag="ps")
                for g in range(NCH):
                    sl = slice(g * W, (g + 1) * W)
                    nc.tensor.matmul(
                        ps2[:, sl], W_I, T[:, sl], start=True, stop=False
                    )
                    nc.tensor.matmul(
                        ps2[:, sl], W_N16, dw[:, sl].bitcast(F16),
                        start=False, stop=True,
                    )
                nc.scalar.copy(bv[:, :], ps2[:, :])
                for g in range(NCH):
                    nc.sync.dma_start(
                        out=out[s, g * CH : (g + 1) * CH, :],
                        in_=bv[:, g * W : (g + 1) * W],
                    )

            tiles = {0: emit_loads(0), 1: emit_loads(1)}
            pendings = []
            for sp in range(0, S, 2):
                if sp + 2 < S:
                    tiles[sp + 2] = emit_loads(sp + 2)
                if sp + 3 < S:
                    tiles[sp + 3] = emit_loads(sp + 3)
                ims = []
                for s in (sp, sp + 1):
                    T, PC = tiles.pop(s)
                    kP = kp.tile([CH, FREE_T], U16, tag="kP")
                    kN = kp.tile([CH, FREE_T], I16, tag="kN")
                    ims.append((T, PC, kP, kN, s))

                dws = {}
                for si, (x, y) in enumerate(SHIFTS):
                    # interleave previous pair's decodes into this pair's shifts
                    if si == 2 and pendings:
                        dws[0] = emit_decode_a(pendings[0][1], pendings[0][2])
                    if si == 4 and pendings:
                        emit_decode_b(pendings[0][0], dws[0], pendings[0][3])
                        dws[1] = emit_decode_a(pendings[1][1], pendings[1][2])
                    if si == 6 and pendings:
                        emit_decode_b(pendings[1][0], dws[1], pendings[1][3])
                        pendings = []
                    for (T, PC, kP, kN, s) in ims:

                        def pc_seg(y_, g_):
                            return PC[:, g_ * SEG + 1 + y_ : g_ * SEG + 1 + y_ + W]

                        h = hp.tile([CH, FREE_T], U16, tag="h")
                        ps = psp.tile([CH, FREE_T], F32, tag="ps")
                        for g in range(NCH):
                            sl = slice(g * W, (g + 1) * W)
                            nc.tensor.matmul(
                                ps[:, sl], W_I, T[:, sl], start=True, stop=False
                            )
                            if x == 0:
                                nc.tensor.matmul(
                                    ps[:, sl], W_N, pc_seg(y, g),
                                    start=False, stop=True,
                                )
                            elif x == -1:
                                nc.tensor.matmul(
                                    ps[:, sl], W_NU, pc_seg(y, g),
                                    start=False, stop=False,
                                )
                                nc.tensor.matmul(
                                    ps[:, sl], W_EU,
                                    pc_seg(y, g - 1) if g > 0 else PADT[:, :],
                                    start=False, stop=True,
                                )
                            else:
                                nc.tensor.matmul(
                                    ps[:, sl], W_ND, pc_seg(y, g),
                                    start=False, stop=False,
                                )
                                nc.tensor.matmul(
                                    ps[:, sl], W_ED,
                                    pc_seg(y, g + 1) if g < NCH - 1 else PADT[:, :],
                                    start=False, stop=True,
                                )
                        nc.scalar.copy(h[:, :].bitcast(F16), ps[:, :])
                        if si == 0:
                            nc.vector.tensor_copy(kP[:, :], h[:, :])
                            nc.vector.tensor_copy(kN[:, :].bitcast(U16), h[:, :])
                        else:
                            nc.vector.tensor_tensor(
                                kP[:, :], h[:, :], kP[:, :], OP.min
                            )
                            nc.vector.tensor_tensor(
                                kN[:, :], h[:, :].bitcast(I16), kN[:, :], OP.min
                            )

                pendings = [(T, kP, kN, s) for (T, PC, kP, kN, s) in ims]

            dw_a = emit_decode_a(pendings[0][1], pendings[0][2])
            emit_decode_b(pendings[0][0], dw_a, pendings[0][3])
            dw_b = emit_decode_a(pendings[1][1], pendings[1][2])
            emit_decode_b(pendings[1][0], dw_b, pendings[1][3])

    nc.finalize()
    return nc


def _get_nc():
    if "nc" not in _CACHE:
        _CACHE["nc"] = _build_nc()
    return _CACHE["nc"]


def kernel(predicted, target, mask=None, _want_results_obj=False, _trace=False):
    """predicted [S,B,H,W], target [B,S,H,W] -> [S,B,H,W] (mask unused)."""
    from concourse.bass_utils import run_bass_kernel_spmd

    nc = _get_nc()
    eye = np.eye(128, dtype=np.float32)
    e_up = np.eye(128, k=1, dtype=np.float32)     # [k, m]=1 at k=m-1
    e_dn = np.eye(128, k=-1, dtype=np.float32)    # [k, m]=1 at k=m+1
    e_bu = np.zeros((128, 128), dtype=np.float32)
    e_bu[127, 0] = 1.0                             # out[0] = rhs[127]
    e_bd = np.zeros((128, 128), dtype=np.float32)
    e_bd[0, 127] = 1.0                             # out[127] = rhs[0]
    eye2 = np.concatenate([eye, -eye, -e_up, -e_dn, -e_bu, -e_bd], axis=1)
    neye16 = np.where(np.eye(128, dtype=bool), 0xBC00, 0).astype(np.uint16)
    in_maps = []
    for b in range(B):
        in_maps.append(
            {
                "pred": np.ascontiguousarray(predicted[:, b]),
                "targ": np.ascontiguousarray(target[b]),
                "eye2": eye2,
                "neye16": neye16,
            }
        )
    res = run_bass_kernel_spmd(nc, in_maps, list(range(B)), trace=_trace)
    outp = np.stack([res.results[b]["out"] for b in range(B)], axis=1)
    if _want_results_obj:
        return outp, res
    return outp


# revision 24
# speedup vs baseline: 1.3629x; 1.2638x over previous
"""Trainium2 Bass kernel for nn_KernelShiftedPrediction (v8: fp16 key scheme,
hoisted prefetch + pipelined decode with PE reconstruct).

Per pixel, over 9 shifts, pick the shifted `predicted` value minimizing
|target - candidate|. Track two fp16 "keys" = fp16(d), d = T - C:

 - kP = unsigned-int16 min over fp16 bits of d (best d>=0 candidate)
 - kN = signed-int16 min over the same bits (best d<0 candidate)

Both are plain 16-bit tensor_tensor mins (2x DVE mode). Decode per
image: compare magnitudes, select winner dw (fp16), reconstruct
C = I@T - I16@dw on the PE, ACT-copy out of PSUM, DMA out.

Software pipeline per iteration s:
  - emit DMA loads for image s+1 first (sync dispatches them before
    any instruction gated on slow producers),
  - 9 shifts of image s: PE d=I@T-I@C (fp32r), ACT fp16 convert
    (PSUM->SBUF), DVE kP/kN min updates,
  - decode of image s-1 interleaved at shift 4 (DVE ops) and shift 6
    (PE reconstruct + ACT out-copy + DMA store).

Accuracy: comparisons are fp16 (and fp32r ~11-bit operand rounding);
near-tie mispicks give rel_err ~1.24e-2 on the contest data (gate
2e-2); off-tie value error ~|d|*2^-11.
"""
import sys

sys.path.insert(0, "/opt/trn_rl_repo")

import numpy as np

S, B, H, W = 10, 8, 512, 512
CH = 128          # chunk rows (partitions)
NCH = H // CH     # 4 segments per image, side by side in the free dim
SEG = W + 2       # per-segment width in padded view tiles
FREE_T = NCH * W      # 2048
FREE_P = NCH * SEG    # 2056
PADVAL = 1.0e30

SHIFTS = [(0, 0), (-1, -1), (-1, 0), (-1, 1), (0, -1), (0, 1), (1, -1),
          (1, 0), (1, 1)]

_CACHE = {}


def _build_nc():
    import concourse.bacc as bacc
    import concourse.mybir as mybir
    from concourse.tile import TileContext

    F32 = mybir.dt.float32
    F32R = mybir.dt.float32r
    F16 = mybir.dt.float16
    U16 = mybir.dt.uint16
    I16 = mybir.dt.int16
    OP = mybir.AluOpType

    nc = bacc.Bacc("TRN2", target_bir_lowering=False, debug=False, num_devices=B)
    pred = nc.declare_dram_parameter("pred", [S, H, W], F32R, isOutput=False)
    targ = nc.declare_dram_parameter("targ", [S, H, W], F32R, isOutput=False)
    eye2 = nc.declare_dram_parameter("eye2", [128, 768], F32R, isOutput=False)
    neye16 = nc.declare_dram_parameter("neye16", [128, 128], U16, isOutput=False)
    out = nc.declare_dram_parameter("out", [S, H, W], F32, isOutput=True)

    with TileContext(nc) as tc:
        with (
            tc.tile_pool(name="cst", bufs=1) as cst,
            tc.tile_pool(name="io", bufs=6) as io,
            tc.tile_pool(name="hp", bufs=4) as hp,
            tc.tile_pool(name="kp", bufs=4) as kp,
            tc.tile_pool(name="dc", bufs=2) as dc,
            tc.tile_pool(name="ps", bufs=4, space="PSUM") as psp,
        ):
            eye = cst.tile([128, 768], F32R)
            nc.sync.dma_start(out=eye[:, :], in_=eye2[:, :])
            W_I = eye[:, 0:128]       # identity
            W_N = eye[:, 128:256]     # -identity
            W_NU = eye[:, 256:384]    # -shifted identity: out[m] = -C[m-1]
            W_ND = eye[:, 384:512]    # -shifted identity: out[m] = -C[m+1]
            W_EU = eye[:, 512:640]    # out[0]   = -rhs[127]
            W_ED = eye[:, 640:768]    # out[127] = -rhs[0]
            ney = cst.tile([128, 128], U16)
            nc.sync.dma_start(out=ney[:, :], in_=neye16[:, :])
            W_N16 = ney[:, :].bitcast(F16)   # -identity fp16
            PADT = cst.tile([CH, W], F32R)
            nc.vector.memset(PADT[:, :].bitcast(F32), PADVAL)

            def emit_loads(s):
                T = io.tile([CH, FREE_T], F32R, tag="T")
                PC = io.tile([CH, FREE_P], F32R, tag="PC")
                ap = PC[:, :].bitcast(F32).rearrange("p (g e) -> p g e", g=NCH)
                nc.gpsimd.memset(ap[:, :, 0:SEG:SEG - 1], PADVAL)
                for g in range(NCH):
                    r0 = g * CH
                    cs = g * SEG + 1
                    nc.sync.dma_start(
                        out=T[:, g * W : (g + 1) * W], in_=targ[s, r0 : r0 + CH, :]
                    )
                    nc.sync.dma_start(
                        out=PC[:, cs : cs + W], in_=pred[s, r0 : r0 + CH, :]
                    )
                return (T, PC)

            def emit_decode_a(kP, kN):
                # dw = magnitude-smaller of kP / kN  (all u16 DVE ops)
                aP = dc.tile([CH, FREE_T], U16, tag="aP")
                aN = dc.tile([CH, FREE_T], U16, tag="aN")
                m = dc.tile([CH, FREE_T], U16, tag="m")
                dw = dc.tile([CH, FREE_T], U16, tag="dw")
                nc.vector.tensor_scalar(
                    out=aP[:, :], in0=kP[:, :], scalar1=0x7FFF, scalar2=None,
                    op0=OP.bitwise_and,
                )
                nc.vector.tensor_scalar(
                    out=aN[:, :], in0=kN[:, :].bitcast(U16), scalar1=0x7FFF,
                    scalar2=None, op0=OP.bitwise_and,
                )
                nc.vector.tensor_tensor(m[:, :], aP[:, :], aN[:, :], OP.is_lt)
                nc.vector.tensor_copy(dw[:, :], kN[:, :].bitcast(U16))
                nc.vector.copy_predicated(dw[:, :], m[:, :], kP[:, :])
                return dw

            def emit_decode_b(T, dw, s):
                # reconstruct C = I@T - I16@dw on PE; ACT copy out; DMA store
                bv = dc.tile([CH, FREE_T], F32, tag="bv")
                for half in range(2):
                    ps2 = psp.tile([CH, FREE_T // 2], F32, tag="ps")
                    for gg in range(2):
                        g = half * 2 + gg
                        sl = slice(g * W, (g + 1) * W)
                        psl = slice(gg * W, (gg + 1) * W)
                        nc.tensor.matmul(
                            ps2[:, psl], W_I, T[:, sl], start=True, stop=False
                        )
                        nc.tensor.matmul(
                            ps2[:, psl], W_N16, dw[:, sl].bitcast(F16),
                            start=False, stop=True,
                        )
                    nc.scalar.copy(
                        bv[:, half * FREE_T // 2 : (half + 1) * FREE_T // 2],
                        ps2[:, :],
                    )
                for g in range(NCH):
                    nc.sync.dma_start(
                        out=out[s, g * CH : (g + 1) * CH, :],
                        in_=bv[:, g * W : (g + 1) * W],
                    )

            tiles = {0: emit_loads(0), 1: emit_loads(1)}
            pendings = []
            for sp in range(0, S, 2):
                if sp + 2 < S:
                    tiles[sp + 2] = emit_loads(sp + 2)
                if sp + 3 < S:
                    tiles[sp + 3] = emit_loads(sp + 3)
                ims = []
                for s in (sp, sp + 1):
                    T, PC = tiles.pop(s)
                    kP = kp.tile([CH, FREE_T], U16, tag="kP")
                    kN = kp.tile([CH, FREE_T], I16, tag="kN")
                    ims.append((T, PC, kP, kN, s))

                dws = {}
                for si, (x, y) in enumerate(SHIFTS):
                    # interleave previous pair's decodes into this pair's shifts
                    if si == 2 and pendings:
                        dws[0] = emit_decode_a(pendings[0][1], pendings[0][2])
                    if si == 4 and pendings:
                        emit_decode_b(pendings[0][0], dws[0], pendings[0][3])
                        dws[1] = emit_decode_a(pendings[1][1], pendings[1][2])
                    if si == 6 and pendings:
                        emit_decode_b(pendings[1][0], dws[1], pendings[1][3])
                        pendings = []
                    for (T, PC, kP, kN, s) in ims:

                        def pc_seg(y_, g_):
                            return PC[:, g_ * SEG + 1 + y_ : g_ * SEG + 1 + y_ + W]

                        h = hp.tile([CH, FREE_T], U16, tag="h")
                        for half in range(2):
                            ps = psp.tile([CH, FREE_T // 2], F32, tag="ps")
                            for gg in range(2):
                                g = half * 2 + gg
                                sl = slice(g * W, (g + 1) * W)
                                psl = slice(gg * W, (gg + 1) * W)
                                nc.tensor.matmul(
                                    ps[:, psl], W_I, T[:, sl],
                                    start=True, stop=False,
                                )
                                if x == 0:
                                    nc.tensor.matmul(
                                        ps[:, psl], W_N, pc_seg(y, g),
                                        start=False, stop=True,
                                    )
                                elif x == -1:
                                    nc.tensor.matmul(
                                        ps[:, psl], W_NU, pc_seg(y, g),
                                        start=False, stop=False,
                                    )
                                    nc.tensor.matmul(
                                        ps[:, psl], W_EU,
                                        pc_seg(y, g - 1) if g > 0 else PADT[:, :],
                                        start=False, stop=True,
                                    )
                                else:
                                    nc.tensor.matmul(
                                        ps[:, psl], W_ND, pc_seg(y, g),
                                        start=False, stop=False,
                                    )
                                    nc.tensor.matmul(
                                        ps[:, psl], W_ED,
                                        pc_seg(y, g + 1) if g < NCH - 1
                                        else PADT[:, :],
                                        start=False, stop=True,
                                    )
                            nc.scalar.copy(
                                h[:, half * FREE_T // 2 : (half + 1) * FREE_T // 2]
                                .bitcast(F16),
                                ps[:, :],
                            )
                        if si == 0:
                            nc.vector.tensor_copy(kP[:, :], h[:, :])
                            nc.vector.tensor_copy(kN[:, :].bitcast(U16), h[:, :])
                        else:
                            nc.vector.tensor_tensor(
                                kP[:, :], h[:, :], kP[:, :], OP.min
                            )
                            nc.vector.tensor_tensor(
                                kN[:, :], h[:, :].bitcast(I16), kN[:, :], OP.min
                            )

                pendings = [(T, kP, kN, s) for (T, PC, kP, kN, s) in ims]

            dw_a = emit_decode_a(pendings[0][1], pendings[0][2])
            emit_decode_b(pendings[0][0], dw_a, pendings[0][3])
            dw_b = emit_decode_a(pendings[1][1], pendings[1][2])
            emit_decode_b(pendings[1][0], dw_b, pendings[1][3])

    nc.finalize()
    return nc


def _get_nc():
    if "nc" not in _CACHE:
        _CACHE["nc"] = _build_nc()
    return _CACHE["nc"]


def kernel(predicted, target, mask=None, _want_results_obj=False, _trace=False):
    """predicted [S,B,H,W], target [B,S,H,W] -> [S,B,H,W] (mask unused)."""
    from concourse.bass_utils import run_bass_kernel_spmd

    nc = _get_nc()
    eye = np.eye(128, dtype=np.float32)
    e_up = np.eye(128, k=1, dtype=np.float32)     # [k, m]=1 at k=m-1
    e_dn = np.eye(128, k=-1, dtype=np.float32)    # [k, m]=1 at k=m+1
    e_bu = np.zeros((128, 128), dtype=np.float32)
    e_bu[127, 0] = 1.0                             # out[0] = rhs[127]
    e_bd = np.zeros((128, 128), dtype=np.float32)
    e_bd[0, 127] = 1.0                             # out[127] = rhs[0]
    eye2 = np.concatenate([eye, -eye, -e_up, -e_dn, -e_bu, -e_bd], axis=1)
    neye16 = np.where(np.eye(128, dtype=bool), 0xBC00, 0).astype(np.uint16)
    in_maps = []
    for b in range(B):
        in_maps.append(
            {
                "pred": np.ascontiguousarray(predicted[:, b]),
                "targ": np.ascontiguousarray(target[b]),
                "eye2": eye2,
                "neye16": neye16,
            }
        )
    res = run_bass_kernel_spmd(nc, in_maps, list(range(B)), trace=_trace)
    outp = np.stack([res.results[b]["out"] for b in range(B)], axis=1)
    if _want_results_obj:
        return outp, res
    return outp
